# revision 5
# baseline (speedup 1.0000x reference)
"""Distributed spectral conv on S2 (SHT -> per-l complex channel mix -> ISHT)
for Trainium2, m-mode sharded across 8 NeuronCores.

Pipeline per core (33 of 257 rfft m-modes per core, zero-padded):
  A: DFT over lon as matmul (data-as-weights)     -> XFT [k, (comp,m,c)]
  B: Legendre transform (contract k)              -> CF  [(comp,i), (l,m)]
  C: per-l complex channel mix (contract i)       -> COUT[(comp,o), (m,l)]
  P1: PE-transpose pivot                          -> OUTT[l, (comp,m,o)]
  D: inverse Legendre (contract l)                -> XKS [k', (comp,m,o)]
  P2: PE-transpose pivot                          -> XK  [(comp,m), (o,k')]
  E: inverse DFT as matmul (contract m-comps)     -> y_part [(o,k'), n]
Host sums the 8 partial y outputs (linear in m-modes).
"""
import numpy as np

import concourse.bass as bass
import concourse.bacc as bacc
import concourse.mybir as mybir
from concourse import tile
from concourse._compat import get_trn_type
from concourse.bass_utils import run_bass_kernel_spmd

F32 = mybir.dt.float32


def _finish(nc):
    return nc

N_CORES = 8
M_LOC = 33            # m modes per core (8*33 = 264 >= 257, rest zero-padded)
MC = 2 * M_LOC        # real+imag components
CIN = 64
COUT_ = 64
NLAT = 256
NLON = 512
MMAX = 257
CK = COUT_ * NLAT     # 16384 output rows
WCHUNK = 16           # l values per weight DMA chunk

_prog_cache = {}


def _build_nc(stages="ABCDE"):
    nc = bacc.Bacc(get_trn_type() or "TRN2", target_bir_lowering=False, debug=False)

    xtb = nc.dram_tensor("xtb", [CIN, NLON, NLAT], F32, kind="ExternalInput")
    fdft = nc.dram_tensor("fdft", [4, 128, MC], F32, kind="ExternalInput")
    shtw_t = nc.dram_tensor("shtw_t", [M_LOC, 2, 128, 256], F32, kind="ExternalInput")
    wmat = nc.dram_tensor("wmat", [64, 256, 3, 64], F32, kind="ExternalInput")
    pct_t = nc.dram_tensor("pct_t", [M_LOC, 2, 128, 256], F32, kind="ExternalInput")
    gdft = nc.dram_tensor("gdft", [MC, NLON], F32, kind="ExternalInput")
    ident = nc.dram_tensor("ident", [128, 128], F32, kind="ExternalInput")
    y_part = nc.dram_tensor("y_part", [CK, NLON], F32, kind="ExternalOutput")

    with tile.TileContext(nc) as tc:
        with tc.tile_pool(name="const", bufs=1) as constp, \
             tc.tile_pool(name="big", bufs=1) as bigp, \
             tc.tile_pool(name="xa", bufs=4) as xap, \
             tc.tile_pool(name="sw", bufs=3) as swp, \
             tc.tile_pool(name="wt", bufs=2) as wtp, \
             tc.tile_pool(name="pt", bufs=3) as ptp, \
             tc.tile_pool(name="ysb", bufs=4) as ysbp, \
             tc.tile_pool(name="ps", bufs=6, space="PSUM") as psp:

            fsb = constp.tile([128, 4, MC], F32)        # [n_in_chunk, nchunk, cm]
            gsb = constp.tile([MC, NLON], F32)
            isb = constp.tile([128, 128], F32)
            nc.sync.dma_start(fsb[:, :, :], fdft.ap().rearrange("a b c -> b a c"))
            nc.sync.dma_start(gsb[:, :], gdft[:, :])
            nc.sync.dma_start(isb[:, :], ident[:, :])

            # ---- stage A: rfft as matmul, data as stationary operand ----
            # XFT[ki, kh*4224 + cm*64 + c] = xf_comp[c, kh*128+ki, m]
            XFT = bigp.tile([128, 2 * MC * 64], F32, tag="bigA")
            XFT_v = XFT.rearrange("p (kh cm c) -> p kh cm c", kh=2, c=64)
            for c in range(CIN):
                xa = xap.tile([128, 4, NLAT], F32)      # [n_in, nchunk, k]
                nc.sync.dma_start(
                    xa[:, :, :], xtb[c].rearrange("(a b) k -> b a k", b=128)
                )
                for kh in range(2):
                    pa = psp.tile([128, MC], F32, tag="ps")
                    for nck in range(4):
                        nc.tensor.matmul(
                            pa[:, :],
                            xa[:, nck, kh * 128:(kh + 1) * 128],
                            fsb[:, nck, :],
                            start=(nck == 0),
                            stop=(nck == 3),
                        )
                    nc.vector.tensor_copy(XFT_v[:, kh, :, c], pa[:, :])

            if "B" not in stages:
                dbg = ysbp.tile([128, NLON], F32)
                nc.vector.tensor_copy(dbg[:, :], XFT[:, 0:NLON])
                nc.sync.dma_start(y_part[0:128, :], dbg[:, :])
                return _finish(nc)
            # ---- stage B: Legendre transform, contract k ----
            # CFr/CFi[i, l*33+m] = coeffs_comp[i, l, m], both at base partitions 0-63
            CFr = bigp.tile([64, 256 * M_LOC], F32, tag="bigB")
            CFi = bigp.tile([64, 256 * M_LOC], F32, tag="bigC")
            CF_vs = [CFr.rearrange("p (l m) -> p l m", m=M_LOC),
                     CFi.rearrange("p (l m) -> p l m", m=M_LOC)]
            for m in range(M_LOC):
                sw = swp.tile([128, 2, 256], F32)       # [ki, kh, l]
                nc.sync.dma_start(
                    sw[:, :, :], shtw_t[m].rearrange("a b c -> b a c")
                )
                for comp in range(2):
                    pb = psp.tile([64, 256], F32, tag="ps")
                    for kh in range(2):
                        nc.tensor.matmul(
                            pb[:, :],
                            XFT_v[:, kh, comp * M_LOC + m, :],
                            sw[:, kh, :],
                            start=(kh == 0),
                            stop=(kh == 1),
                        )
                    nc.vector.tensor_copy(CF_vs[comp][:, :, m], pb[:, :])

            if "C" not in stages:
                dbg = ysbp.tile([128, NLON], F32)
                nc.vector.tensor_copy(dbg[0:64, :], CFr[:, 0:NLON])
                nc.vector.tensor_copy(dbg[64:128, :], CFi[:, 0:NLON])
                nc.sync.dma_start(y_part[0:128, :], dbg[:, :])
                return _finish(nc)
            # ---- stage C: per-l complex channel mixing, contract i ----
            # COUT[comp*64+o, m*256+l] = out_comp[o, l, m]
            COUT = bigp.tile([128, M_LOC * 256], F32, tag="bigA")
            COUT_v = COUT.rearrange("p (m l) -> p m l", l=256)
            for ci in range(256 // WCHUNK):
                wt = wtp.tile([64, WCHUNK, 3, 64], F32)
                src = wmat.ap()[:, ci * WCHUNK:(ci + 1) * WCHUNK, :, :]
                nc.sync.dma_start(wt[:, :, :, :], src)
                for lj in range(WCHUNK):
                    l = ci * WCHUNK + lj
                    por = psp.tile([64, M_LOC], F32, tag="ps")
                    poi = psp.tile([64, M_LOC], F32, tag="ps")
                    cf_r = CF_vs[0][:, l, :]
                    cf_i = CF_vs[1][:, l, :]
                    # or = wr.T cr - wi.T ci ; oi = wi.T cr + wr.T ci
                    nc.tensor.matmul(por[:, :], wt[:, lj, 0, :], cf_r, start=True, stop=False)
                    nc.tensor.matmul(por[:, :], wt[:, lj, 2, :], cf_i, start=False, stop=True)
                    nc.tensor.matmul(poi[:, :], wt[:, lj, 1, :], cf_r, start=True, stop=False)
                    nc.tensor.matmul(poi[:, :], wt[:, lj, 0, :], cf_i, start=False, stop=True)
                    nc.vector.tensor_copy(COUT_v[0:64, :, l], por[:, :])
                    nc.vector.tensor_copy(COUT_v[64:128, :, l], poi[:, :])

            if "P1" not in stages and "D" not in stages:
                dbg = ysbp.tile([128, NLON], F32)
                nc.vector.tensor_copy(dbg[:, :], COUT[:, 0:NLON])
                nc.sync.dma_start(y_part[0:128, :], dbg[:, :])
                return _finish(nc)
            # ---- pivot P1: COUT -> OUTT[l, (comp,m,o)] via PE transpose ----
            OUTT = bigp.tile([128, 2 * MC * 64], F32, tag="bigB")
            OUTT_v = OUTT.rearrange("p (lc cm o) -> p lc cm o", lc=2, o=64)
            for comp in range(2):
                for m in range(M_LOC):
                    for lc in range(2):
                        ptr = psp.tile([128, 64], F32, tag="ps")
                        nc.tensor.transpose(
                            ptr[:, :],
                            COUT_v[comp * 64:(comp + 1) * 64, m, lc * 128:(lc + 1) * 128],
                            isb[comp * 64:(comp + 1) * 64, comp * 64:(comp + 1) * 64],
                        )
                        nc.vector.tensor_copy(
                            OUTT_v[:, lc, comp * M_LOC + m, :], ptr[:, :]
                        )

            if "D" not in stages:
                dbg = ysbp.tile([128, NLON], F32)
                nc.vector.tensor_copy(dbg[:, :], OUTT[:, 0:NLON])
                nc.sync.dma_start(y_part[0:128, :], dbg[:, :])
                return _finish(nc)
            # ---- stage D: inverse Legendre, contract l ----
            # XKS[ki', kc*4224 + cm*64 + o] = xk_comp[o, kc*128+ki', m]
            XKS = bigp.tile([128, 2 * MC * 64], F32, tag="bigA")
            XKS_v = XKS.rearrange("p (kc cm o) -> p kc cm o", kc=2, o=64)
            for m in range(M_LOC):
                pt = ptp.tile([128, 2, 256], F32)       # [li, lc, kp]
                nc.sync.dma_start(
                    pt[:, :, :], pct_t[m].rearrange("a b c -> b a c")
                )
                for comp in range(2):
                    for kc in range(2):
                        pd = psp.tile([128, 64], F32, tag="ps")
                        for lc in range(2):
                            nc.tensor.matmul(
                                pd[:, :],
                                pt[:, lc, kc * 128:(kc + 1) * 128],
                                OUTT_v[:, lc, comp * M_LOC + m, :],
                                start=(lc == 0),
                                stop=(lc == 1),
                            )
                        nc.vector.tensor_copy(
                            XKS_v[:, kc, comp * M_LOC + m, :], pd[:, :]
                        )

            if "P2" not in stages and "E" not in stages:
                dbg = ysbp.tile([128, NLON], F32)
                nc.vector.tensor_copy(dbg[:, :], XKS[:, 0:NLON])
                nc.sync.dma_start(y_part[0:128, :], dbg[:, :])
                return _finish(nc)
            # ---- pivot P2: XKS -> XK[(comp,m), (o,k')] via PE transpose ----
            XK = bigp.tile([MC, CK], F32, tag="bigB")
            XK_v = XK.rearrange("p (o k) -> p o k", k=NLAT)
            for o in range(64):
                for kc in range(2):
                    pt2 = psp.tile([MC, 128], F32, tag="ps")
                    nc.tensor.transpose(
                        pt2[:, :], XKS_v[:, kc, :, o], isb[:, :]
                    )
                    nc.vector.tensor_copy(
                        XK_v[:, o, kc * 128:(kc + 1) * 128], pt2[:, :]
                    )

            if "E" not in stages:
                dbg = ysbp.tile([MC, NLON], F32)
                nc.vector.tensor_copy(dbg[:, :], XK[:, 0:NLON])
                nc.sync.dma_start(y_part[0:MC, :], dbg[:, :])
                return _finish(nc)
            # ---- stage E: inverse DFT as matmul, contract m-comps ----
            for j in range(CK // 128):
                pe = psp.tile([128, NLON], F32, tag="ps")
                nc.tensor.matmul(
                    pe[:, :], XK[:, j * 128:(j + 1) * 128], gsb[:, :],
                    start=True, stop=True,
                )
                ys = ysbp.tile([128, NLON], F32)
                nc.vector.tensor_copy(ys[:, :], pe[:, :])
                nc.sync.dma_start(y_part[j * 128:(j + 1) * 128, :], ys[:, :])

    return _finish(nc)


def _get_nc(stages="ABCDE"):
    if stages not in _prog_cache:
        nc = _build_nc(stages)
        nc.compile()
        _prog_cache[stages] = nc
    return _prog_cache[stages]


def _core_ms(r):
    return [r * M_LOC + j for j in range(M_LOC) if r * M_LOC + j < MMAX]


def make_in_maps(x, weight_r, weight_i, pct, sht_w):
    x = np.asarray(x, dtype=np.float32)
    wr = np.asarray(weight_r, dtype=np.float32)[0]          # [i, o, l]
    wi = np.asarray(weight_i, dtype=np.float32)[0]
    pct = np.asarray(pct, dtype=np.float32)                 # [m, l, k]
    sht_w = np.asarray(sht_w, dtype=np.float32)

    xtb = np.ascontiguousarray(x[0].transpose(0, 2, 1))     # [c, n, k]
    # wmat[i, l, t, o]: t0 = wr, t1 = wi, t2 = -wi
    wmat = np.empty((64, 256, 3, 64), np.float32)
    wmat[:, :, 0, :] = wr.transpose(0, 2, 1)
    wmat[:, :, 1, :] = wi.transpose(0, 2, 1)
    wmat[:, :, 2, :] = -wi.transpose(0, 2, 1)
    ident = np.eye(128, dtype=np.float32)

    n = np.arange(NLON)
    in_maps = []
    for r in range(N_CORES):
        ms = _core_ms(r)
        nm = len(ms)
        marr = np.array(ms)

        ang = 2.0 * np.pi * marr[None, :] * n[:, None] / NLON   # [n, nm]
        fdft = np.zeros((NLON, MC), np.float32)
        fdft[:, :nm] = (2.0 * np.pi / NLON) * np.cos(ang)
        fdft[:, M_LOC:M_LOC + nm] = -(2.0 * np.pi / NLON) * np.sin(ang)
        fdft = fdft.reshape(4, 128, MC)

        cmf = np.where((marr == 0) | (marr == NLON // 2), 1.0, 2.0)
        gdft = np.zeros((MC, NLON), np.float32)
        gdft[:nm, :] = cmf[:, None] * np.cos(ang.T)
        gdft[M_LOC:M_LOC + nm, :] = -cmf[:, None] * np.sin(ang.T)

        shtw_t = np.zeros((M_LOC, 2, 128, 256), np.float32)
        # shtw_t[j, kh, ki, l] = sht_w[m_j, l, kh*128+ki]
        shtw_t[:nm] = sht_w[marr].transpose(0, 2, 1).reshape(nm, 2, 128, 256)

        pct_t = np.zeros((M_LOC, 2, 128, 256), np.float32)
        # pct_t[j, lc, li, kp] = pct[m_j, lc*128+li, kp]
        pct_t[:nm] = pct[marr].reshape(nm, 2, 128, 256)

        in_maps.append({
            "xtb": xtb, "fdft": np.ascontiguousarray(fdft),
            "shtw_t": shtw_t, "wmat": wmat, "pct_t": pct_t,
            "gdft": gdft, "ident": ident,
        })
    return in_maps


def kernel(x, weight_r, weight_i, pct, sht_w):
    x_np = np.asarray(x)
    nc = _get_nc()
    in_maps = make_in_maps(x_np, weight_r, weight_i, pct, sht_w)
    res = run_bass_kernel_spmd(nc, in_maps, list(range(N_CORES)))
    y = np.zeros((CK, NLON), np.float64)
    for r in range(N_CORES):
        y += np.asarray(res.results[r]["y_part"], dtype=np.float64)
    y = y.astype(np.float32).reshape(1, COUT_, NLAT, NLON)
    return (y, x_np)


# revision 6
# speedup vs baseline: 1.0164x; 1.0164x over previous
"""Distributed spectral conv on S2 (SHT -> per-l complex channel mix -> ISHT)
for Trainium2, m-mode sharded across 8 NeuronCores.

Pipeline per core (33 of 257 rfft m-modes per core, zero-padded):
  A: DFT over lon as matmul (data-as-weights)     -> XFT [k, (comp,m,c)]
  B: Legendre transform (contract k)              -> CF  [(comp,i), (l,m)]
  C: per-l complex channel mix (contract i)       -> COUT[(comp,o), (m,l)]
  P1: PE-transpose pivot                          -> OUTT[l, (comp,m,o)]
  D: inverse Legendre (contract l)                -> XKS [k', (comp,m,o)]
  P2: PE-transpose pivot                          -> XK  [(comp,m), (o,k')]
  E: inverse DFT as matmul (contract m-comps)     -> y_part [(o,k'), n]
Host sums the 8 partial y outputs (linear in m-modes).
"""
import numpy as np

import concourse.bass as bass
import concourse.bacc as bacc
import concourse.mybir as mybir
from concourse import tile
from concourse._compat import get_trn_type
from concourse.bass_utils import run_bass_kernel_spmd

F32 = mybir.dt.float32
F32R = mybir.dt.float32r


def _finish(nc):
    return nc

N_CORES = 8
M_LOC = 33            # m modes per core (8*33 = 264 >= 257, rest zero-padded)
MC = 2 * M_LOC        # real+imag components
CIN = 64
COUT_ = 64
NLAT = 256
NLON = 512
MMAX = 257
CK = COUT_ * NLAT     # 16384 output rows
WCHUNK = 16           # l values per weight DMA chunk

_prog_cache = {}


def _build_nc(stages="ABCDE"):
    nc = bacc.Bacc(get_trn_type() or "TRN2", target_bir_lowering=False, debug=False)

    xtb = nc.dram_tensor("xtb", [CIN, NLON, NLAT], F32, kind="ExternalInput")
    fdft = nc.dram_tensor("fdft", [4, 128, MC], F32, kind="ExternalInput")
    shtw_t = nc.dram_tensor("shtw_t", [M_LOC, 2, 128, 256], F32, kind="ExternalInput")
    wmat = nc.dram_tensor("wmat", [64, 256, 3, 64], F32, kind="ExternalInput")
    pct_t = nc.dram_tensor("pct_t", [M_LOC, 2, 128, 256], F32, kind="ExternalInput")
    gdft = nc.dram_tensor("gdft", [MC, NLON], F32, kind="ExternalInput")
    ident = nc.dram_tensor("ident", [128, 128], F32, kind="ExternalInput")
    y_part = nc.dram_tensor("y_part", [CK, NLON], F32, kind="ExternalOutput")

    with tile.TileContext(nc) as tc:
        with tc.tile_pool(name="const", bufs=1) as constp, \
             tc.tile_pool(name="big", bufs=1) as bigp, \
             tc.tile_pool(name="xa", bufs=4) as xap, \
             tc.tile_pool(name="sw", bufs=3) as swp, \
             tc.tile_pool(name="wt", bufs=2) as wtp, \
             tc.tile_pool(name="pt", bufs=3) as ptp, \
             tc.tile_pool(name="ysb", bufs=4) as ysbp, \
             tc.tile_pool(name="ps", bufs=6, space="PSUM") as psp:

            fsb = constp.tile([128, 4, MC], F32)        # [n_in_chunk, nchunk, cm]
            gsb = constp.tile([MC, NLON], F32)
            isb = constp.tile([128, 128], F32)
            nc.sync.dma_start(fsb[:, :, :], fdft.ap().rearrange("a b c -> b a c"))
            nc.sync.dma_start(gsb[:, :], gdft[:, :])
            nc.sync.dma_start(isb[:, :], ident[:, :])

            # ---- stage A: rfft as matmul, data as stationary operand ----
            # XFT[ki, kh*4224 + cm*64 + c] = xf_comp[c, kh*128+ki, m]
            XFT = bigp.tile([128, 2 * MC * 64], F32R, tag="bigA")
            XFT_v = XFT.rearrange("p (kh cm c) -> p kh cm c", kh=2, c=64)
            for c in range(CIN):
                xa = xap.tile([128, 4, NLAT], F32)      # [n_in, nchunk, k]
                nc.sync.dma_start(
                    xa[:, :, :], xtb[c].rearrange("(a b) k -> b a k", b=128)
                )
                for kh in range(2):
                    pa = psp.tile([128, MC], F32, tag="ps")
                    for nck in range(4):
                        nc.tensor.matmul(
                            pa[:, :],
                            xa[:, nck, kh * 128:(kh + 1) * 128],
                            fsb[:, nck, :],
                            start=(nck == 0),
                            stop=(nck == 3),
                        )
                    nc.vector.tensor_copy(XFT_v[:, kh, :, c], pa[:, :])

            if "B" not in stages:
                dbg = ysbp.tile([128, NLON], F32)
                nc.vector.tensor_copy(dbg[:, :], XFT[:, 0:NLON].bitcast(F32))
                nc.sync.dma_start(y_part[0:128, :], dbg[:, :])
                return _finish(nc)
            # ---- stage B: Legendre transform, contract k ----
            # CFr/CFi[i, l*33+m] = coeffs_comp[i, l, m], both at base partitions 0-63
            CFr = bigp.tile([64, 256 * M_LOC], F32, tag="bigB")
            CFi = bigp.tile([64, 256 * M_LOC], F32, tag="bigC")
            CF_vs = [CFr.rearrange("p (l m) -> p l m", m=M_LOC),
                     CFi.rearrange("p (l m) -> p l m", m=M_LOC)]
            for m in range(M_LOC):
                sw = swp.tile([128, 2, 256], F32)       # [ki, kh, l]
                nc.sync.dma_start(
                    sw[:, :, :], shtw_t[m].rearrange("a b c -> b a c")
                )
                swr = swp.tile([128, 2, 256], F32R, tag="swr")
                nc.vector.tensor_copy(swr[:, :, :], sw[:, :, :])
                for comp in range(2):
                    pb = psp.tile([64, 256], F32, tag="ps")
                    for kh in range(2):
                        nc.tensor.matmul(
                            pb[:, :],
                            XFT_v[:, kh, comp * M_LOC + m, :],
                            swr[:, kh, :],
                            start=(kh == 0),
                            stop=(kh == 1),
                        )
                    nc.vector.tensor_copy(CF_vs[comp][:, :, m], pb[:, :])

            if "C" not in stages:
                dbg = ysbp.tile([128, NLON], F32)
                nc.vector.tensor_copy(dbg[0:64, :], CFr[:, 0:NLON])
                nc.vector.tensor_copy(dbg[64:128, :], CFi[:, 0:NLON])
                nc.sync.dma_start(y_part[0:128, :], dbg[:, :])
                return _finish(nc)
            # ---- stage C: per-l complex channel mixing, contract i ----
            # COUT[comp*64+o, m*256+l] = out_comp[o, l, m]
            COUT = bigp.tile([128, M_LOC * 256], F32, tag="bigA")
            COUT_v = COUT.rearrange("p (m l) -> p m l", l=256)
            for ci in range(256 // WCHUNK):
                wt = wtp.tile([64, WCHUNK, 3, 64], F32)
                src = wmat.ap()[:, ci * WCHUNK:(ci + 1) * WCHUNK, :, :]
                nc.sync.dma_start(wt[:, :, :, :], src)
                for lj in range(WCHUNK):
                    l = ci * WCHUNK + lj
                    por = psp.tile([64, M_LOC], F32, tag="ps")
                    poi = psp.tile([64, M_LOC], F32, tag="ps")
                    cf_r = CF_vs[0][:, l, :]
                    cf_i = CF_vs[1][:, l, :]
                    # or = wr.T cr - wi.T ci ; oi = wi.T cr + wr.T ci
                    nc.tensor.matmul(por[:, :], wt[:, lj, 0, :], cf_r, start=True, stop=False)
                    nc.tensor.matmul(por[:, :], wt[:, lj, 2, :], cf_i, start=False, stop=True)
                    nc.tensor.matmul(poi[:, :], wt[:, lj, 1, :], cf_r, start=True, stop=False)
                    nc.tensor.matmul(poi[:, :], wt[:, lj, 0, :], cf_i, start=False, stop=True)
                    nc.vector.tensor_copy(COUT_v[0:64, :, l], por[:, :])
                    nc.vector.tensor_copy(COUT_v[64:128, :, l], poi[:, :])

            if "P1" not in stages and "D" not in stages:
                dbg = ysbp.tile([128, NLON], F32)
                nc.vector.tensor_copy(dbg[:, :], COUT[:, 0:NLON])
                nc.sync.dma_start(y_part[0:128, :], dbg[:, :])
                return _finish(nc)
            # ---- pivot P1: COUT -> OUTT[l, (comp,m,o)] via PE transpose ----
            OUTT = bigp.tile([128, 2 * MC * 64], F32, tag="bigB")
            OUTT_v = OUTT.rearrange("p (lc cm o) -> p lc cm o", lc=2, o=64)
            for comp in range(2):
                for m in range(M_LOC):
                    for lc in range(2):
                        ptr = psp.tile([128, 64], F32, tag="ps")
                        nc.tensor.transpose(
                            ptr[:, :],
                            COUT_v[comp * 64:(comp + 1) * 64, m, lc * 128:(lc + 1) * 128],
                            isb[comp * 64:(comp + 1) * 64, comp * 64:(comp + 1) * 64],
                        )
                        nc.vector.tensor_copy(
                            OUTT_v[:, lc, comp * M_LOC + m, :], ptr[:, :]
                        )

            if "D" not in stages:
                dbg = ysbp.tile([128, NLON], F32)
                nc.vector.tensor_copy(dbg[:, :], OUTT[:, 0:NLON])
                nc.sync.dma_start(y_part[0:128, :], dbg[:, :])
                return _finish(nc)
            # ---- stage D: inverse Legendre, contract l ----
            # XKS[ki', kc*4224 + cm*64 + o] = xk_comp[o, kc*128+ki', m]
            XKS = bigp.tile([128, 2 * MC * 64], F32, tag="bigA")
            XKS_v = XKS.rearrange("p (kc cm o) -> p kc cm o", kc=2, o=64)
            for m in range(M_LOC):
                pt = ptp.tile([128, 2, 256], F32)       # [li, lc, kp]
                nc.sync.dma_start(
                    pt[:, :, :], pct_t[m].rearrange("a b c -> b a c")
                )
                for comp in range(2):
                    for kc in range(2):
                        pd = psp.tile([128, 64], F32, tag="ps")
                        for lc in range(2):
                            nc.tensor.matmul(
                                pd[:, :],
                                pt[:, lc, kc * 128:(kc + 1) * 128],
                                OUTT_v[:, lc, comp * M_LOC + m, :],
                                start=(lc == 0),
                                stop=(lc == 1),
                            )
                        nc.vector.tensor_copy(
                            XKS_v[:, kc, comp * M_LOC + m, :], pd[:, :]
                        )

            if "P2" not in stages and "E" not in stages:
                dbg = ysbp.tile([128, NLON], F32)
                nc.vector.tensor_copy(dbg[:, :], XKS[:, 0:NLON])
                nc.sync.dma_start(y_part[0:128, :], dbg[:, :])
                return _finish(nc)
            # ---- pivot P2: XKS -> XK[(comp,m), (o,k')] via PE transpose ----
            XK = bigp.tile([MC, CK], F32R, tag="bigB")
            XK_v = XK.rearrange("p (o k) -> p o k", k=NLAT)
            for o in range(64):
                for kc in range(2):
                    pt2 = psp.tile([MC, 128], F32, tag="ps")
                    nc.tensor.transpose(
                        pt2[:, :], XKS_v[:, kc, :, o], isb[:, :]
                    )
                    nc.vector.tensor_copy(
                        XK_v[:, o, kc * 128:(kc + 1) * 128], pt2[:, :]
                    )

            if "E" not in stages:
                dbg = ysbp.tile([MC, NLON], F32)
                nc.vector.tensor_copy(dbg[:, :], XK[:, 0:NLON].bitcast(F32))
                nc.sync.dma_start(y_part[0:MC, :], dbg[:, :])
                return _finish(nc)
            # ---- stage E: inverse DFT as matmul, contract m-comps ----
            gsbr = constp.tile([MC, NLON], F32R)
            nc.vector.tensor_copy(gsbr[:, :], gsb[:, :])
            for j in range(CK // 128):
                pe = psp.tile([128, NLON], F32, tag="ps")
                nc.tensor.matmul(
                    pe[:, :], XK[:, j * 128:(j + 1) * 128], gsbr[:, :],
                    start=True, stop=True,
                )
                ys = ysbp.tile([128, NLON], F32)
                nc.vector.tensor_copy(ys[:, :], pe[:, :])
                nc.sync.dma_start(y_part[j * 128:(j + 1) * 128, :], ys[:, :])

    return _finish(nc)


def _get_nc(stages="ABCDE"):
    if stages not in _prog_cache:
        nc = _build_nc(stages)
        nc.compile()
        _prog_cache[stages] = nc
    return _prog_cache[stages]


def _core_ms(r):
    return [r * M_LOC + j for j in range(M_LOC) if r * M_LOC + j < MMAX]


def make_in_maps(x, weight_r, weight_i, pct, sht_w):
    x = np.asarray(x, dtype=np.float32)
    wr = np.asarray(weight_r, dtype=np.float32)[0]          # [i, o, l]
    wi = np.asarray(weight_i, dtype=np.float32)[0]
    pct = np.asarray(pct, dtype=np.float32)                 # [m, l, k]
    sht_w = np.asarray(sht_w, dtype=np.float32)

    xtb = np.ascontiguousarray(x[0].transpose(0, 2, 1))     # [c, n, k]
    # wmat[i, l, t, o]: t0 = wr, t1 = wi, t2 = -wi
    wmat = np.empty((64, 256, 3, 64), np.float32)
    wmat[:, :, 0, :] = wr.transpose(0, 2, 1)
    wmat[:, :, 1, :] = wi.transpose(0, 2, 1)
    wmat[:, :, 2, :] = -wi.transpose(0, 2, 1)
    ident = np.eye(128, dtype=np.float32)

    n = np.arange(NLON)
    in_maps = []
    for r in range(N_CORES):
        ms = _core_ms(r)
        nm = len(ms)
        marr = np.array(ms)

        ang = 2.0 * np.pi * marr[None, :] * n[:, None] / NLON   # [n, nm]
        fdft = np.zeros((NLON, MC), np.float32)
        fdft[:, :nm] = (2.0 * np.pi / NLON) * np.cos(ang)
        fdft[:, M_LOC:M_LOC + nm] = -(2.0 * np.pi / NLON) * np.sin(ang)
        fdft = fdft.reshape(4, 128, MC)

        cmf = np.where((marr == 0) | (marr == NLON // 2), 1.0, 2.0)
        gdft = np.zeros((MC, NLON), np.float32)
        gdft[:nm, :] = cmf[:, None] * np.cos(ang.T)
        gdft[M_LOC:M_LOC + nm, :] = -cmf[:, None] * np.sin(ang.T)

        shtw_t = np.zeros((M_LOC, 2, 128, 256), np.float32)
        # shtw_t[j, kh, ki, l] = sht_w[m_j, l, kh*128+ki]
        shtw_t[:nm] = sht_w[marr].transpose(0, 2, 1).reshape(nm, 2, 128, 256)

        pct_t = np.zeros((M_LOC, 2, 128, 256), np.float32)
        # pct_t[j, lc, li, kp] = pct[m_j, lc*128+li, kp]
        pct_t[:nm] = pct[marr].reshape(nm, 2, 128, 256)

        in_maps.append({
            "xtb": xtb, "fdft": np.ascontiguousarray(fdft),
            "shtw_t": shtw_t, "wmat": wmat, "pct_t": pct_t,
            "gdft": gdft, "ident": ident,
        })
    return in_maps


def kernel(x, weight_r, weight_i, pct, sht_w):
    x_np = np.asarray(x)
    nc = _get_nc()
    in_maps = make_in_maps(x_np, weight_r, weight_i, pct, sht_w)
    res = run_bass_kernel_spmd(nc, in_maps, list(range(N_CORES)))
    y = np.zeros((CK, NLON), np.float64)
    for r in range(N_CORES):
        y += np.asarray(res.results[r]["y_part"], dtype=np.float64)
    y = y.astype(np.float32).reshape(1, COUT_, NLAT, NLON)
    return (y, x_np)


# revision 9
# speedup vs baseline: 1.5435x; 1.5187x over previous
"""Distributed spectral conv on S2 (SHT -> per-l complex channel mix -> ISHT)
for Trainium2, m-mode sharded across 8 NeuronCores.

Pipeline per core (33 of 257 rfft m-modes per core, zero-padded):
  A: DFT over lon, basis-as-weights fp32r N=512   -> psum [66cm, 512ck]
  T1: PE-transpose pivot                          -> XFT [k, (comp,m,c)] f32r
  B: Legendre transform fp32r (contract k)        -> CFr/CFi [i, (l,m)] bf16
  C: per-l channel mix, paired weights bf16       -> COUT3 [(comp,m), (l,o)] bf16
  P1: PE-transpose pivot                          -> OUTT [l, (m,comp,o)] bf16
  D: inverse Legendre bf16 (contract l)           -> XKS [k', (m,comp,o)] bf16
  P2: PE-transpose pivot                          -> XK [(m,comp), (o,k')] bf16
  E: inverse DFT bf16 (contract m-comps)          -> y_part [(o,k'), n] f32
Host sums the 8 partial y outputs (linear in m-modes).
"""
import numpy as np
import ml_dtypes

import concourse.bass as bass
import concourse.bacc as bacc
import concourse.mybir as mybir
from concourse import tile
from concourse._compat import get_trn_type
from concourse.bass_utils import run_bass_kernel_spmd

F32 = mybir.dt.float32
F32R = mybir.dt.float32r
BF16 = mybir.dt.bfloat16

N_CORES = 8
M_LOC = 33            # m modes per core (8*33 = 264 >= 257, rest zero-padded)
MC = 2 * M_LOC        # real+imag components
CIN = 64
COUT_ = 64
NLAT = 256
NLON = 512
MMAX = 257
CK = COUT_ * NLAT     # 16384 output rows
WCHUNK = 16           # l values per weight DMA chunk

_prog_cache = {}


def _build_nc(stages="ABCDE"):
    nc = bacc.Bacc(get_trn_type() or "TRN2", target_bir_lowering=False, debug=False)

    xt = nc.dram_tensor("xt", [4, 128, CK], F32, kind="ExternalInput")
    fdft = nc.dram_tensor("fdft", [4, 128, MC], F32, kind="ExternalInput")
    shtw_t = nc.dram_tensor("shtw_t", [M_LOC, 2, 128, 256], F32, kind="ExternalInput")
    wpair = nc.dram_tensor("wpair", [64, 256, 2, 64], BF16, kind="ExternalInput")
    pct_t = nc.dram_tensor("pct_t", [M_LOC, 2, 128, 256], BF16, kind="ExternalInput")
    gdft = nc.dram_tensor("gdft", [MC, NLON], BF16, kind="ExternalInput")
    ident = nc.dram_tensor("ident", [128, 128], F32, kind="ExternalInput")
    y_part = nc.dram_tensor("y_part", [CK, NLON], F32, kind="ExternalOutput")

    with tile.TileContext(nc) as tc:
        with tc.tile_pool(name="const", bufs=1) as constp, \
             tc.tile_pool(name="big", bufs=1) as bigp, \
             tc.tile_pool(name="xa", bufs=6) as xap, \
             tc.tile_pool(name="xf2", bufs=3) as xf2p, \
             tc.tile_pool(name="sw", bufs=3) as swp, \
             tc.tile_pool(name="wt", bufs=2) as wtp, \
             tc.tile_pool(name="pt", bufs=3) as ptp, \
             tc.tile_pool(name="ysb", bufs=4) as ysbp, \
             tc.tile_pool(name="ps", bufs=6, space="PSUM") as psp:

            fsb = constp.tile([128, 4, MC], F32)        # [n_in_chunk, nchunk, cm]
            fsbr = constp.tile([128, 4, MC], F32R)
            gsb = constp.tile([MC, NLON], BF16)
            isb = constp.tile([128, 128], F32)
            isbb = constp.tile([128, 128], BF16)
            nc.sync.dma_start(fsb[:, :, :], fdft.ap().rearrange("a b c -> b a c"))
            nc.vector.tensor_copy(fsbr[:, :, :], fsb[:, :, :])
            nc.sync.dma_start(gsb[:, :], gdft[:, :])
            nc.sync.dma_start(isb[:, :], ident[:, :])
            nc.vector.tensor_copy(isbb[:, :], isb[:, :])

            # ---- stage A: DFT as matmul, basis stationary, fp32r ----
            # then T1 transposes into XFT[ki, kh*4224 + cm*64 + c]
            XFT = bigp.tile([128, 2 * MC * 64], F32R, tag="bigA")
            XFT_v = XFT.rearrange("p (kh cm c) -> p kh cm c", kh=2, c=64)
            for span in range(16):          # 1024 ck-columns per span
                xar = []
                for nc4 in range(4):
                    xa = xap.tile([128, 1024], F32, tag="xa")
                    nc.sync.dma_start(
                        xa[:, :], xt[nc4, :, span * 1024:(span + 1) * 1024]
                    )
                    xr = xap.tile([128, 1024], F32R, tag="xar")
                    nc.vector.tensor_copy(xr[:, :], xa[:, :])
                    xar.append(xr)
                for sub in range(2):
                    c = span * 4 + sub * 2  # chunk covers channels c, c+1
                    pa = psp.tile([MC, 512], F32, tag="ps")
                    for nc4 in range(4):
                        nc.tensor.matmul(
                            pa[:, :],
                            fsbr[:, nc4, :],
                            xar[nc4][:, sub * 512:(sub + 1) * 512],
                            start=(nc4 == 0),
                            stop=(nc4 == 3),
                        )
                    xf2 = xf2p.tile([MC, 512], F32)
                    nc.vector.tensor_copy(xf2[:, :], pa[:, :])
                    for q in range(4):
                        cc = c + q // 2
                        kh = q % 2
                        ptr = psp.tile([128, MC], F32, tag="ps")
                        nc.tensor.transpose(
                            ptr[:, :], xf2[:, q * 128:(q + 1) * 128], isb[:MC, :MC]
                        )
                        nc.vector.tensor_copy(XFT_v[:, kh, :, cc], ptr[:, :])

            if "B" not in stages:
                dbg = ysbp.tile([128, NLON], F32)
                nc.vector.tensor_copy(dbg[:, :], XFT[:, 0:NLON].bitcast(F32))
                nc.sync.dma_start(y_part[0:128, :], dbg[:, :])
                return nc
            # ---- stage B: Legendre transform fp32r, contract k -> bf16 coeffs ----
            # CFP1[i, l, mm] = [cfr | cfi], CFP2 = [-cfi | cfr]  (mm: 0-32 / 33-65)
            CFP1 = bigp.tile([64, 256 * MC], BF16, tag="bigB")
            CFP2 = bigp.tile([64, 256 * MC], BF16, tag="bigC")
            CFP1_v = CFP1.rearrange("p (l mm) -> p l mm", mm=MC)
            CFP2_v = CFP2.rearrange("p (l mm) -> p l mm", mm=MC)
            for m in range(M_LOC):
                sw = swp.tile([128, 2, 256], F32)       # [ki, kh, l]
                nc.sync.dma_start(
                    sw[:, :, :], shtw_t[m].rearrange("a b c -> b a c")
                )
                swr = swp.tile([128, 2, 256], F32R, tag="swr")
                nc.vector.tensor_copy(swr[:, :, :], sw[:, :, :])
                for comp in range(2):
                    pb = psp.tile([64, 256], F32, tag="ps")
                    for kh in range(2):
                        nc.tensor.matmul(
                            pb[:, :],
                            XFT_v[:, kh, comp * M_LOC + m, :],
                            swr[:, kh, :],
                            start=(kh == 0),
                            stop=(kh == 1),
                        )
                    if comp == 0:   # cfr
                        nc.vector.tensor_copy(CFP1_v[:, :, m], pb[:, :])
                        nc.vector.tensor_copy(CFP2_v[:, :, M_LOC + m], pb[:, :])
                    else:           # cfi
                        nc.vector.tensor_copy(CFP1_v[:, :, M_LOC + m], pb[:, :])
                        nc.scalar.mul(CFP2_v[:, :, m], pb[:, :], -1.0)

            if "C" not in stages:
                dbg = ysbp.tile([128, NLON], F32)
                nc.vector.tensor_copy(dbg[0:64, :], CFP1[:, 0:NLON])
                nc.vector.tensor_copy(dbg[64:128, :], CFP2[:, 0:NLON])
                nc.sync.dma_start(y_part[0:128, :], dbg[:, :])
                return nc
            # ---- stage C: per-l channel mix, paired-column lhsT, bf16 ----
            # psum [66 (or;oi), 64 o] = [cfr|cfi].T @ wr + [-cfi|cfr].T @ wi
            # COUT3[comp*33+m, l*64+o]
            COUT3 = bigp.tile([MC, 256 * 64], BF16, tag="bigA")
            COUT3_v = COUT3.rearrange("p (l o) -> p l o", o=64)
            for ci in range(256 // WCHUNK):
                wt = wtp.tile([64, WCHUNK, 2, 64], BF16)
                nc.sync.dma_start(
                    wt[:, :, :, :], wpair.ap()[:, ci * WCHUNK:(ci + 1) * WCHUNK, :, :]
                )
                for lj in range(WCHUNK):
                    l = ci * WCHUNK + lj
                    pc = psp.tile([MC, 64], F32, tag="ps")
                    nc.tensor.matmul(pc[:, :], CFP1_v[:, l, :], wt[:, lj, 0, :],
                                     start=True, stop=False)
                    nc.tensor.matmul(pc[:, :], CFP2_v[:, l, :], wt[:, lj, 1, :],
                                     start=False, stop=True)
                    nc.vector.tensor_copy(COUT3_v[:, l, :], pc[:, :])

            if "P1" not in stages and "D" not in stages:
                dbg = ysbp.tile([MC, NLON], F32)
                nc.vector.tensor_copy(dbg[:, :], COUT3[:, 0:NLON])
                nc.sync.dma_start(y_part[0:MC, :], dbg[:, :])
                return nc
            # ---- pivot P1: COUT3 -> OUTT[l, (m,comp,o)] via PE transpose ----
            OUTT = bigp.tile([128, 2 * MC * 64], BF16, tag="bigB")
            OUTT_v = OUTT.rearrange("p (lc m cp o) -> p lc m cp o", lc=2, cp=2, o=64)
            for o in range(64):
                for lc in range(2):
                    ptr = psp.tile([128, MC], BF16, tag="ps")
                    nc.tensor.transpose(
                        ptr[:, :], COUT3_v[:, lc * 128:(lc + 1) * 128, o], isbb[:MC, :MC]
                    )
                    # psum cols are (comp, m); dest traversal comp-outer, m-inner
                    nc.vector.tensor_copy(
                        OUTT_v[:, lc, :, :, o].rearrange("p m cp -> p cp m"),
                        ptr.rearrange("p (cp m) -> p cp m", cp=2),
                    )

            if "D" not in stages:
                dbg = ysbp.tile([128, NLON], F32)
                nc.vector.tensor_copy(dbg[:, :], OUTT[:, 0:NLON])
                nc.sync.dma_start(y_part[0:128, :], dbg[:, :])
                return nc
            # ---- stage D: inverse Legendre bf16, contract l ----
            # XKS[ki', kc*4224 + (m*2+comp)*64 + o] = xk[o, kc*128+ki', m, comp]
            XKS = bigp.tile([128, 2 * MC * 64], BF16, tag="bigA")
            XKS_v = XKS.rearrange("p (kc m cp o) -> p kc m cp o", kc=2, cp=2, o=64)
            for m in range(M_LOC):
                pt = ptp.tile([128, 2, 256], BF16)      # [li, lc, kp]
                nc.sync.dma_start(
                    pt[:, :, :], pct_t[m].rearrange("a b c -> b a c")
                )
                for kc in range(2):
                    pd = psp.tile([128, 128], F32, tag="ps")
                    for lc in range(2):
                        nc.tensor.matmul(
                            pd[:, :],
                            pt[:, lc, kc * 128:(kc + 1) * 128],
                            OUTT_v[:, lc, m, :, :],
                            start=(lc == 0),
                            stop=(lc == 1),
                        )
                    nc.vector.tensor_copy(XKS_v[:, kc, m, :, :], pd[:, :])

            if "P2" not in stages and "E" not in stages:
                dbg = ysbp.tile([128, NLON], F32)
                nc.vector.tensor_copy(dbg[:, :], XKS[:, 0:NLON])
                nc.sync.dma_start(y_part[0:128, :], dbg[:, :])
                return nc
            # ---- pivot P2: XKS -> XK[(m,comp), (o,k')] via PE transpose ----
            XK = bigp.tile([MC, CK], BF16, tag="bigB")
            XK_v = XK.rearrange("p (o k) -> p o k", k=NLAT)
            for o in range(64):
                for kc in range(2):
                    pt2 = psp.tile([MC, 128], BF16, tag="ps")
                    nc.tensor.transpose(
                        pt2[:, :], XKS_v[:, kc, :, :, o], isbb[:, :]
                    )
                    nc.vector.tensor_copy(
                        XK_v[:, o, kc * 128:(kc + 1) * 128], pt2[:, :]
                    )

            if "E" not in stages:
                dbg = ysbp.tile([MC, NLON], F32)
                nc.vector.tensor_copy(dbg[:, :], XK[:, 0:NLON])
                nc.sync.dma_start(y_part[0:MC, :], dbg[:, :])
                return nc
            # ---- stage E: inverse DFT as matmul bf16, contract m-comps ----
            for j in range(CK // 128):
                pe = psp.tile([128, NLON], F32, tag="ps")
                nc.tensor.matmul(
                    pe[:, :], XK[:, j * 128:(j + 1) * 128], gsb[:, :],
                    start=True, stop=True,
                )
                ys = ysbp.tile([128, NLON], F32)
                nc.vector.tensor_copy(ys[:, :], pe[:, :])
                nc.sync.dma_start(y_part[j * 128:(j + 1) * 128, :], ys[:, :])

    return nc


def _get_nc(stages="ABCDE"):
    if stages not in _prog_cache:
        nc = _build_nc(stages)
        nc.compile()
        _prog_cache[stages] = nc
    return _prog_cache[stages]


def _core_ms(r):
    return [r * M_LOC + j for j in range(M_LOC) if r * M_LOC + j < MMAX]


def make_in_maps(x, weight_r, weight_i, pct, sht_w):
    x = np.asarray(x, dtype=np.float32)
    wr = np.asarray(weight_r, dtype=np.float32)[0]          # [i, o, l]
    wi = np.asarray(weight_i, dtype=np.float32)[0]
    pct = np.asarray(pct, dtype=np.float32)                 # [m, l, k]
    sht_w = np.asarray(sht_w, dtype=np.float32)

    # xt[nc4, ni, ck] with n = nc4*128 + ni, ck = c*256 + k
    xt = np.ascontiguousarray(
        x[0].reshape(CK, NLON).T.reshape(4, 128, CK)
    )
    # wpair[i, l, v, :]: v0 = wr, v1 = wi
    wpair = np.empty((64, 256, 2, 64), ml_dtypes.bfloat16)
    wpair[:, :, 0, :] = wr.transpose(0, 2, 1)
    wpair[:, :, 1, :] = wi.transpose(0, 2, 1)
    ident = np.eye(128, dtype=np.float32)

    n = np.arange(NLON)
    in_maps = []
    for r in range(N_CORES):
        ms = _core_ms(r)
        nm = len(ms)
        marr = np.array(ms)

        ang = 2.0 * np.pi * marr[None, :] * n[:, None] / NLON   # [n, nm]
        fdft = np.zeros((NLON, MC), np.float32)
        fdft[:, :nm] = (2.0 * np.pi / NLON) * np.cos(ang)
        fdft[:, M_LOC:M_LOC + nm] = -(2.0 * np.pi / NLON) * np.sin(ang)
        fdft = fdft.reshape(4, 128, MC)

        cmf = np.where((marr == 0) | (marr == NLON // 2), 1.0, 2.0)
        # gdft rows INTERLEAVED (m, comp): row 2j = c cos, row 2j+1 = -c sin
        gdft = np.zeros((MC, NLON), np.float32)
        gdft[0:2 * nm:2, :] = cmf[:, None] * np.cos(ang.T)
        gdft[1:2 * nm:2, :] = -cmf[:, None] * np.sin(ang.T)

        shtw_t = np.zeros((M_LOC, 2, 128, 256), np.float32)
        shtw_t[:nm] = sht_w[marr].transpose(0, 2, 1).reshape(nm, 2, 128, 256)

        pct_t = np.zeros((M_LOC, 2, 128, 256), np.float32)
        pct_t[:nm] = pct[marr].reshape(nm, 2, 128, 256)

        in_maps.append({
            "xt": xt, "fdft": np.ascontiguousarray(fdft),
            "shtw_t": shtw_t, "wpair": wpair,
            "pct_t": pct_t.astype(ml_dtypes.bfloat16),
            "gdft": gdft.astype(ml_dtypes.bfloat16), "ident": ident,
        })
    return in_maps


def kernel(x, weight_r, weight_i, pct, sht_w):
    x_np = np.asarray(x)
    nc = _get_nc()
    in_maps = make_in_maps(x_np, weight_r, weight_i, pct, sht_w)
    res = run_bass_kernel_spmd(nc, in_maps, list(range(N_CORES)))
    y = np.zeros((CK, NLON), np.float64)
    for r in range(N_CORES):
        y += np.asarray(res.results[r]["y_part"], dtype=np.float64)
    y = y.astype(np.float32).reshape(1, COUT_, NLAT, NLON)
    return (y, x_np)


# revision 10
# speedup vs baseline: 1.5516x; 1.0053x over previous
"""Distributed spectral conv on S2 (SHT -> per-l complex channel mix -> ISHT)
for Trainium2, m-mode sharded across 8 NeuronCores.

Pipeline per core (33 of 257 rfft m-modes per core, zero-padded):
  A: DFT over lon, basis-as-weights fp32r N=512   -> psum [66cm, 512ck]
  T1: PE-transpose pivot                          -> XFT [k, (comp,m,c)] f32r
  B: Legendre transform fp32r (contract k)        -> CFQ1/CFQ2 [(s,i), (j,cm)] bf16
  C: per-l-pair channel mix, block-diag bf16      -> COUT4 [o, (l,cm)] bf16
  P1: PE-transpose pivot                          -> OUTT [l, (m,comp,o)] bf16
  D: inverse Legendre bf16 (contract l)           -> XKS [k', (m,comp,o)] bf16
  P2: PE-transpose pivot                          -> XK [(m,comp), (o,k')] bf16
  E: inverse DFT bf16 (contract m-comps)          -> y_part [(o,k'), n] f32
Host sums the 8 partial y outputs (linear in m-modes).
"""
import numpy as np
import ml_dtypes

import concourse.bass as bass
import concourse.bacc as bacc
import concourse.mybir as mybir
from concourse import tile
from concourse._compat import get_trn_type
from concourse.bass_utils import run_bass_kernel_spmd

F32 = mybir.dt.float32
F32R = mybir.dt.float32r
BF16 = mybir.dt.bfloat16

N_CORES = 8
M_LOC = 33            # m modes per core (8*33 = 264 >= 257, rest zero-padded)
MC = 2 * M_LOC        # real+imag components
CIN = 64
COUT_ = 64
NLAT = 256
NLON = 512
MMAX = 257
CK = COUT_ * NLAT     # 16384 output rows
WCHUNK = 8            # j-pairs per weight DMA chunk

_prog_cache = {}


def _build_nc(stages="ABCDE"):
    nc = bacc.Bacc(get_trn_type() or "TRN2", target_bir_lowering=False, debug=False)

    xt = nc.dram_tensor("xt", [4, 128, CK], F32R, kind="ExternalInput")
    fdft = nc.dram_tensor("fdft", [4, 128, MC], F32R, kind="ExternalInput")
    shtw_t = nc.dram_tensor("shtw_t", [M_LOC, 2, 128, 256], F32R, kind="ExternalInput")
    wblk = nc.dram_tensor("wblk", [128, 2, 128, 128], BF16, kind="ExternalInput")
    pct_t = nc.dram_tensor("pct_t", [M_LOC, 2, 128, 256], BF16, kind="ExternalInput")
    gdft = nc.dram_tensor("gdft", [MC, NLON], BF16, kind="ExternalInput")
    ident = nc.dram_tensor("ident", [128, 128], F32, kind="ExternalInput")
    y_part = nc.dram_tensor("y_part", [CK, NLON], F32, kind="ExternalOutput")

    with tile.TileContext(nc) as tc:
        with tc.tile_pool(name="const", bufs=1) as constp, \
             tc.tile_pool(name="big", bufs=1) as bigp, \
             tc.tile_pool(name="xa", bufs=6) as xap, \
             tc.tile_pool(name="xf2", bufs=3) as xf2p, \
             tc.tile_pool(name="sw", bufs=3) as swp, \
             tc.tile_pool(name="wt", bufs=2) as wtp, \
             tc.tile_pool(name="pt", bufs=3) as ptp, \
             tc.tile_pool(name="ysb", bufs=6) as ysbp, \
             tc.tile_pool(name="ps", bufs=6, space="PSUM") as psp:

            fsbr = constp.tile([128, 4, MC], F32R)      # [n_in_chunk, nchunk, cm]
            gsb = constp.tile([MC, NLON], BF16)
            isb = constp.tile([128, 128], F32)
            isbb = constp.tile([128, 128], BF16)
            nc.sync.dma_start(fsbr[:, :, :], fdft.ap().rearrange("a b c -> b a c"))
            nc.sync.dma_start(gsb[:, :], gdft[:, :])
            nc.sync.dma_start(isb[:, :], ident[:, :])
            nc.vector.tensor_copy(isbb[:, :], isb[:, :])

            # ---- stage A: DFT as matmul, basis stationary, fp32r ----
            # then T1 transposes into XFT[ki, kh*4224 + cm*64 + c]
            XFT = bigp.tile([128, 2 * MC * 64], F32R, tag="bigA")
            XFT_v = XFT.rearrange("p (kh cm c) -> p kh cm c", kh=2, c=64)
            for span in range(16):          # 1024 ck-columns per span
                xar = []
                for nc4 in range(4):
                    xr = xap.tile([128, 1024], F32R, tag="xar")
                    nc.sync.dma_start(
                        xr[:, :], xt[nc4, :, span * 1024:(span + 1) * 1024]
                    )
                    xar.append(xr)
                for sub in range(2):
                    c = span * 4 + sub * 2  # chunk covers channels c, c+1
                    pa = psp.tile([MC, 512], F32, tag="ps")
                    for nc4 in range(4):
                        nc.tensor.matmul(
                            pa[:, :],
                            fsbr[:, nc4, :],
                            xar[nc4][:, sub * 512:(sub + 1) * 512],
                            start=(nc4 == 0),
                            stop=(nc4 == 3),
                        )
                    xf2 = xf2p.tile([MC, 512], F32)
                    if sub == 0:
                        nc.vector.tensor_copy(xf2[:, :], pa[:, :])
                    else:
                        nc.scalar.copy(xf2[:, :], pa[:, :])
                    for q in range(4):
                        cc = c + q // 2
                        kh = q % 2
                        ptr = psp.tile([128, MC], F32, tag="ps")
                        nc.tensor.transpose(
                            ptr[:, :], xf2[:, q * 128:(q + 1) * 128], isb[:MC, :MC]
                        )
                        nc.vector.tensor_copy(XFT_v[:, kh, :, cc], ptr[:, :])

            if "B" not in stages:
                dbg = ysbp.tile([128, NLON], F32)
                nc.vector.tensor_copy(dbg[:, :], XFT[:, 0:NLON].bitcast(F32))
                nc.sync.dma_start(y_part[0:128, :], dbg[:, :])
                return nc
            # ---- stage B: Legendre fp32r -> block-layout bf16 coeffs ----
            # CFQ1[(s,i), (j, cm)] = cf_comp[i, l=2j+s, m] with cm=(comp,m)
            # CFQ2: cols (0:33)=-cfi, (33:66)=cfr
            CFQ1 = bigp.tile([128, 128 * MC], BF16, tag="bigB")
            CFQ2 = bigp.tile([128, 128 * MC], BF16, tag="bigC")
            CFQ1_v = CFQ1.rearrange("p (j cm) -> p j cm", cm=MC)
            CFQ2_v = CFQ2.rearrange("p (j cm) -> p j cm", cm=MC)
            for m in range(M_LOC):
                swr = swp.tile([128, 2, 256], F32R)     # [ki, kh, l]
                nc.sync.dma_start(
                    swr[:, :, :], shtw_t[m].rearrange("a b c -> b a c")
                )
                for comp in range(2):
                    pb = psp.tile([64, 256], F32, tag="ps")
                    for kh in range(2):
                        nc.tensor.matmul(
                            pb[:, :],
                            XFT_v[:, kh, comp * M_LOC + m, :],
                            swr[:, kh, :],
                            start=(kh == 0),
                            stop=(kh == 1),
                        )
                    pbv = pb.rearrange("p (j s) -> p j s", s=2)
                    if comp == 0:   # cfr -> CFQ1 col m, CFQ2 col 33+m
                        for s in range(2):
                            nc.vector.tensor_copy(
                                CFQ1_v[s * 64:(s + 1) * 64, :, m], pbv[:, :, s])
                            nc.vector.tensor_copy(
                                CFQ2_v[s * 64:(s + 1) * 64, :, M_LOC + m], pbv[:, :, s])
                    else:           # cfi -> CFQ1 col 33+m, -cfi -> CFQ2 col m
                        for s in range(2):
                            nc.vector.tensor_copy(
                                CFQ1_v[s * 64:(s + 1) * 64, :, M_LOC + m], pbv[:, :, s])
                            nc.scalar.mul(
                                CFQ2_v[s * 64:(s + 1) * 64, :, m], pbv[:, :, s], -1.0)

            if "C" not in stages:
                dbg = ysbp.tile([128, NLON], F32)
                nc.vector.tensor_copy(dbg[:, :], CFQ1[:, 0:NLON])
                nc.sync.dma_start(y_part[0:128, :], dbg[:, :])
                return nc
            # ---- stage C: channel mix, block-diag per l-pair, bf16 ----
            # psum [128 (s,o), 66 cm] = Wblk_r.T @ CFQ1[:, j, :] + Wblk_i.T @ CFQ2[:, j, :]
            # COUT4[o, l*66 + cm]
            COUT4 = bigp.tile([64, 256 * MC], BF16, tag="bigA")
            COUT4_v = COUT4.rearrange("p (l cm) -> p l cm", cm=MC)
            for ci in range(128 // WCHUNK):
                wt = wtp.tile([128, WCHUNK, 2, 128], BF16)
                nc.sync.dma_start(
                    wt[:, :, :, :], wblk.ap()[ci * WCHUNK:(ci + 1) * WCHUNK, :, :, :]
                    .rearrange("j v a b -> a j v b")
                )
                for jj in range(WCHUNK):
                    j = ci * WCHUNK + jj
                    pc = psp.tile([128, MC], F32, tag="ps")
                    nc.tensor.matmul(pc[:, :], wt[:, jj, 0, :], CFQ1_v[:, j, :],
                                     start=True, stop=False)
                    nc.tensor.matmul(pc[:, :], wt[:, jj, 1, :], CFQ2_v[:, j, :],
                                     start=False, stop=True)
                    nc.vector.tensor_copy(COUT4_v[:, 2 * j, :], pc[0:64, :])
                    nc.vector.tensor_copy(COUT4_v[:, 2 * j + 1, :], pc[64:128, :])

            if "P1" not in stages and "D" not in stages:
                dbg = ysbp.tile([64, NLON], F32)
                nc.vector.tensor_copy(dbg[:, :], COUT4[:, 0:NLON])
                nc.sync.dma_start(y_part[0:64, :], dbg[:, :])
                return nc
            # ---- pivot P1: COUT4 -> OUTT[l, (m,comp,o)] via PE transpose ----
            OUTT = bigp.tile([128, 2 * MC * 64], BF16, tag="bigB")
            OUTT_v = OUTT.rearrange("p (lc m cp o) -> p lc m cp o", lc=2, cp=2, o=64)
            for cm in range(MC):
                cp, m = cm // M_LOC, cm % M_LOC
                for lc in range(2):
                    ptr = psp.tile([128, 64], BF16, tag="ps")
                    nc.tensor.transpose(
                        ptr[:, :],
                        COUT4_v[:, lc * 128:(lc + 1) * 128, cm], isbb[:64, :64]
                    )
                    nc.vector.tensor_copy(OUTT_v[:, lc, m, cp, :], ptr[:, :])

            if "D" not in stages:
                dbg = ysbp.tile([128, NLON], F32)
                nc.vector.tensor_copy(dbg[:, :], OUTT[:, 0:NLON])
                nc.sync.dma_start(y_part[0:128, :], dbg[:, :])
                return nc
            # ---- stage D: inverse Legendre bf16, contract l ----
            # XKS[ki', kc*4224 + (m*2+comp)*64 + o]
            XKS = bigp.tile([128, 2 * MC * 64], BF16, tag="bigA")
            XKS_v = XKS.rearrange("p (kc m cp o) -> p kc m cp o", kc=2, cp=2, o=64)
            for m in range(M_LOC):
                pt = ptp.tile([128, 2, 256], BF16)      # [li, lc, kp]
                nc.sync.dma_start(
                    pt[:, :, :], pct_t[m].rearrange("a b c -> b a c")
                )
                for kc in range(2):
                    pd = psp.tile([128, 128], F32, tag="ps")
                    for lc in range(2):
                        nc.tensor.matmul(
                            pd[:, :],
                            pt[:, lc, kc * 128:(kc + 1) * 128],
                            OUTT_v[:, lc, m, :, :],
                            start=(lc == 0),
                            stop=(lc == 1),
                        )
                    if m % 2 == 0:
                        nc.vector.tensor_copy(XKS_v[:, kc, m, :, :], pd[:, :])
                    else:
                        nc.scalar.copy(XKS_v[:, kc, m, :, :], pd[:, :])

            if "P2" not in stages and "E" not in stages:
                dbg = ysbp.tile([128, NLON], F32)
                nc.vector.tensor_copy(dbg[:, :], XKS[:, 0:NLON])
                nc.sync.dma_start(y_part[0:128, :], dbg[:, :])
                return nc
            # ---- pivot P2: XKS -> XK[(m,comp), (o,k')] via PE transpose ----
            XK = bigp.tile([MC, CK], BF16, tag="bigB")
            XK_v = XK.rearrange("p (o k) -> p o k", k=NLAT)
            for o in range(64):
                for kc in range(2):
                    pt2 = psp.tile([MC, 128], BF16, tag="ps")
                    nc.tensor.transpose(
                        pt2[:, :], XKS_v[:, kc, :, :, o], isbb[:, :]
                    )
                    nc.vector.tensor_copy(
                        XK_v[:, o, kc * 128:(kc + 1) * 128], pt2[:, :]
                    )

            if "E" not in stages:
                dbg = ysbp.tile([MC, NLON], F32)
                nc.vector.tensor_copy(dbg[:, :], XK[:, 0:NLON])
                nc.sync.dma_start(y_part[0:MC, :], dbg[:, :])
                return nc
            # ---- stage E: inverse DFT as matmul bf16, contract m-comps ----
            for j in range(CK // 128):
                pe = psp.tile([128, NLON], F32, tag="ps")
                nc.tensor.matmul(
                    pe[:, :], XK[:, j * 128:(j + 1) * 128], gsb[:, :],
                    start=True, stop=True,
                )
                ys = ysbp.tile([128, NLON], F32)
                if j % 2 == 0:
                    nc.vector.tensor_copy(ys[:, :], pe[:, :])
                else:
                    nc.scalar.copy(ys[:, :], pe[:, :])
                nc.sync.dma_start(y_part[j * 128:(j + 1) * 128, :], ys[:, :])

    return nc


def _get_nc(stages="ABCDE"):
    if stages not in _prog_cache:
        nc = _build_nc(stages)
        nc.compile()
        _prog_cache[stages] = nc
    return _prog_cache[stages]


def _core_ms(r):
    return [r * M_LOC + j for j in range(M_LOC) if r * M_LOC + j < MMAX]


def make_in_maps(x, weight_r, weight_i, pct, sht_w):
    x = np.asarray(x, dtype=np.float32)
    wr = np.asarray(weight_r, dtype=np.float32)[0]          # [i, o, l]
    wi = np.asarray(weight_i, dtype=np.float32)[0]
    pct = np.asarray(pct, dtype=np.float32)                 # [m, l, k]
    sht_w = np.asarray(sht_w, dtype=np.float32)

    # xt[nc4, ni, ck] with n = nc4*128 + ni, ck = c*256 + k
    xt = np.ascontiguousarray(
        x[0].reshape(CK, NLON).T.reshape(4, 128, CK)
    )
    # wblk[j, v, (s,i), (s,o)]: block-diag of w_v[:, :, 2j+s], v0=wr v1=wi
    wblk = np.zeros((128, 2, 128, 128), ml_dtypes.bfloat16)
    wblk[:, 0, 0:64, 0:64] = wr[:, :, 0::2].transpose(2, 0, 1)
    wblk[:, 0, 64:128, 64:128] = wr[:, :, 1::2].transpose(2, 0, 1)
    wblk[:, 1, 0:64, 0:64] = wi[:, :, 0::2].transpose(2, 0, 1)
    wblk[:, 1, 64:128, 64:128] = wi[:, :, 1::2].transpose(2, 0, 1)
    ident = np.eye(128, dtype=np.float32)

    n = np.arange(NLON)
    in_maps = []
    for r in range(N_CORES):
        ms = _core_ms(r)
        nm = len(ms)
        marr = np.array(ms)

        ang = 2.0 * np.pi * marr[None, :] * n[:, None] / NLON   # [n, nm]
        fdft = np.zeros((NLON, MC), np.float32)
        fdft[:, :nm] = (2.0 * np.pi / NLON) * np.cos(ang)
        fdft[:, M_LOC:M_LOC + nm] = -(2.0 * np.pi / NLON) * np.sin(ang)
        fdft = fdft.reshape(4, 128, MC)

        cmf = np.where((marr == 0) | (marr == NLON // 2), 1.0, 2.0)
        # gdft rows INTERLEAVED (m, comp): row 2j = c cos, row 2j+1 = -c sin
        gdft = np.zeros((MC, NLON), np.float32)
        gdft[0:2 * nm:2, :] = cmf[:, None] * np.cos(ang.T)
        gdft[1:2 * nm:2, :] = -cmf[:, None] * np.sin(ang.T)

        shtw_t = np.zeros((M_LOC, 2, 128, 256), np.float32)
        shtw_t[:nm] = sht_w[marr].transpose(0, 2, 1).reshape(nm, 2, 128, 256)

        pct_t = np.zeros((M_LOC, 2, 128, 256), np.float32)
        pct_t[:nm] = pct[marr].reshape(nm, 2, 128, 256)

        in_maps.append({
            "xt": xt, "fdft": np.ascontiguousarray(fdft),
            "shtw_t": shtw_t, "wblk": wblk,
            "pct_t": pct_t.astype(ml_dtypes.bfloat16),
            "gdft": gdft.astype(ml_dtypes.bfloat16), "ident": ident,
        })
    return in_maps


def kernel(x, weight_r, weight_i, pct, sht_w):
    x_np = np.asarray(x)
    nc = _get_nc()
    in_maps = make_in_maps(x_np, weight_r, weight_i, pct, sht_w)
    res = run_bass_kernel_spmd(nc, in_maps, list(range(N_CORES)))
    y = np.zeros((CK, NLON), np.float64)
    for r in range(N_CORES):
        y += np.asarray(res.results[r]["y_part"], dtype=np.float64)
    y = y.astype(np.float32).reshape(1, COUT_, NLAT, NLON)
    return (y, x_np)
